# revision 1
# baseline (speedup 1.0000x reference)
"""Canny edge detector on 8 Trainium2 NeuronCores (Bass/Tile).

Sharding: row slabs. Core i owns output rows [118*i, 118*(i+1)) of ALL 8
images. (The reference's flat gather at B=8 cross-wires images inside NMS:
sel_pos(b,h,w) = dirconv_b(gm_{idx(b,h,w)})(h,w), so every output pixel needs
all 8 images' gradient-magnitude maps at its rows -> shard by rows, not by
image.) The leftover band (rows 944..1023) is computed per-image on the
owning core and the gm maps are exchanged through DRAM collectives
(AllGather for plain maps, AllToAll for reader-shift-specific maps).

All compute-engine APs must start at partition 0 (HW constraint), so row
re-alignment between pipeline stages is done with SBUF->SBUF DMAs.
"""

import os

# Tile's subtile dependency tracking emits >1 embedded sync-wait on
# S2S2D2_STT instructions, which the ISA encoding cannot hold ("Too many
# sync wait commands" in codegen). With whole-tile deps the wait-absorbing
# guard ops keep every STT at <=1 embedded wait.
os.environ.setdefault("BY_DEFAULT_DISABLE_SUBTILE_DEPS", "1")

import numpy as np

H = 1024
W = 1024
B = 8
NC = 8
SLAB = 118                    # main-slab output rows per core
B8_START = SLAB * NC          # 944
B8_ROWS = H - B8_START        # 80
LOW_T, HIGH_T = 2.5, 5.0
T22SQ = float(np.float32(np.tan(np.pi / 8.0)) ** 2)

# direction index -> (dr, dc) neighbor offset of dir_f channel d
DELTAS = {0: (0, 1), 1: (1, 1), 2: (1, 0), 3: (1, -1),
          4: (0, -1), 5: (-1, -1), 6: (-1, 0), 7: (-1, 1)}


def _gauss5():
    n = np.arange(5, dtype=np.float32) - 2.0
    return np.exp(-0.5 * n * n).astype(np.float32)


def _band(n_in, n_out, offset, taps):
    """M[k, m] = taps[k - m - offset] for k-m-offset in range(len(taps))."""
    m_ = np.zeros((n_in, n_out), np.float32)
    for mm in range(n_out):
        for t, w in enumerate(taps):
            k = mm + offset + t
            if 0 <= k < n_in:
                m_[k, mm] = w
    return m_


def _const_mats(core):
    g = _gauss5()
    g0 = float(g[0])
    mats = {}
    # main slab: x/hb tile row k <-> img row a+k, a = 118i-5
    # bl row m <-> img a+2+m (124 rows); BV[k,m] = g0*g[k-m]
    mats["BV"] = _band(128, 124, 0, (g0 * g).tolist())
    # gx/gy row m <-> img a+3+m = 118i-2+m (122 rows); bl k <-> a+2+k
    b121 = _band(124, 122, 0, [1.0, 2.0, 1.0])
    b10m1 = _band(124, 122, 0, [1.0, 0.0, -1.0])
    if core == 0:  # img rows -2,-1 must yield gm=0 (zero-pad semantics)
        b121[:, 0:2] = 0.0
        b10m1[:, 0:2] = 0.0
    mats["B121"] = b121
    mats["B121N"] = -b121
    mats["B10M1"] = b10m1
    mats["B10M1X2"] = 2.0 * b10m1
    # strong row k (base 0) <-> thin img row 118i-1+k
    # mp row p <-> img 118i-1+p (col 0 dummy); needs strong k = p-1,p,p+1
    bones = _band(120, 119, -1, [1.0, 1.0, 1.0])
    bones[:, 0] = 0.0
    if core == 0:
        bones[:, 1] = 0.0  # border row 0
    mats["BONES"] = bones
    # B8 block: x8 row k <-> img 936+k; bl8 row m <-> img 938+m (86 rows)
    mats["BV8"] = _band(88, 86, 0, (g0 * g).tolist())
    # gx8 row m <-> img 940+m (84 rows); bl8 k <-> 938+k: band k-m in {1,2,3}
    b121_8 = _band(86, 84, 1, [1.0, 2.0, 1.0])
    b10m1_8 = _band(86, 84, 1, [1.0, 0.0, -1.0])
    mats["B121_8"] = b121_8
    mats["B121N_8"] = -b121_8
    mats["B10M1_8"] = b10m1_8
    mats["B10M1X2_8"] = 2.0 * b10m1_8
    # strong8 row k (base 0) <-> img 943+k; mp8 row p <-> img 943+p
    # (col 0 dummy); needs strong8 k = p-1,p,p+1
    bones8 = _band(81, 81, -1, [1.0, 1.0, 1.0])
    bones8[:, 0] = 0.0
    bones8[:, 80] = 0.0  # border row 1023
    mats["BONES8"] = bones8
    return {k: np.ascontiguousarray(v, np.float32) for k, v in mats.items()}


MAT_SPECS = {
    "BV": [128, 124], "B121": [124, 122], "B121N": [124, 122],
    "B10M1": [124, 122], "B10M1X2": [124, 122], "BONES": [120, 119],
    "BV8": [88, 86], "B121_8": [86, 84], "B121N_8": [86, 84],
    "B10M1_8": [86, 84], "B10M1X2_8": [86, 84], "BONES8": [81, 81],
}

_CACHE = {}


def _build_program():
    if "nc" in _CACHE:
        return _CACHE["nc"]
    import concourse.bass as bass
    import concourse.mybir as mybir
    from concourse.tile import TileContext

    f32 = mybir.dt.float32
    bf16 = mybir.dt.bfloat16
    u8 = mybir.dt.uint8
    Alu = mybir.AluOpType

    g = _gauss5()
    r01 = float(g[0] / g[1])
    r12 = float(g[1] / g[2])
    r21 = float(g[2] / g[1])
    r10 = float(g[1] / g[0])

    nc = bass.Bass()

    def guard(out_ap, in0_ap, in1_ap):
        # Obsolete: _legalize_waits() NoOp-splits any multi-wait instruction
        # after scheduling, which is cheaper than extra DVE data ops.
        pass

    def fence(t):
        pass

    xm = nc.declare_dram_parameter("xm", [B * 3, 128, W], f32, isOutput=False)
    x8 = nc.declare_dram_parameter("x8", [3, 88, W], f32, isOutput=False)
    mat_d = {k: nc.declare_dram_parameter(k, v, f32, isOutput=False)
             for k, v in MAT_SPECS.items()}
    outm = nc.declare_dram_parameter("outm", [B, SLAB, W], f32, isOutput=True)
    out8 = nc.declare_dram_parameter("out8", [B8_ROWS, W], f32, isOutput=True)

    with TileContext(nc) as tc:
        with (
            tc.tile_pool(name="consts", bufs=1) as cpool,
            tc.tile_pool(name="gmp", bufs=1) as gmpool,
            tc.tile_pool(name="msk", bufs=1) as mskpool,
            tc.tile_pool(name="dram", bufs=1, space="DRAM") as dpool,
        ):
            mt = {}
            for name, shp in MAT_SPECS.items():
                t = cpool.tile(shp, f32, tag=name)
                nc.sync.dma_start(out=t[:], in_=mat_d[name][:])
                mt[name] = t

            gm_tiles = []
            masks = []
            # =========== conv phase (own scoped pools) =====================
            with (
                tc.tile_pool(name="xin", bufs=3) as xpool,
                tc.tile_pool(name="hbt", bufs=2) as hbpool,
                tc.tile_pool(name="bls", bufs=2) as blspool,
                tc.tile_pool(name="sq", bufs=2) as sqpool,
                tc.tile_pool(name="gsum", bufs=2) as gsumpool,
                tc.tile_pool(name="mskt", bufs=2) as msktpool,
                tc.tile_pool(name="psA", bufs=2, space="PSUM") as psA,
                tc.tile_pool(name="psB", bufs=1, space="PSUM") as psB,
            ):
                def conv_pipeline(xt, n_in, bv, b121, b121n, b10m1, b10m1x2,
                                  gm_acc, gxs, gys, c, n_bl, n_gxy,
                                  pe_hblur=False):
                    bl = psA.tile([n_bl, W], f32, tag="bl")
                    if pe_hblur:
                        # full 2D blur as 5 shifted-column accumulated
                        # streams: bl = sum_h (g_h * band(g)) @ x<<(h-2)
                        lhs5 = [bv, mt["BVG1"], mt["BVG2"], mt["BVG1"], bv]
                        for lo in (0, 512):
                            for h in range(5):
                                nc.tensor.matmul(
                                    out=bl[:, lo:lo + 512],
                                    lhsT=lhs5[h][0:n_in, 0:n_bl],
                                    rhs=xt[:, h + lo:h + lo + 512],
                                    start=(h == 0), stop=(h == 4))
                    else:
                        # H-blur (Horner, 4 fused ops) -> h2 [n_in, W]
                        h1 = hbpool.tile([n_in, W], f32, tag="h1")
                        h2 = hbpool.tile([n_in, W], f32, tag="h2")
                        guard(h1, xt, h2)
                        guard(h2, xt, h1)
                        nc.vector.scalar_tensor_tensor(
                            out=h1[:], in0=xt[:, 0:W], scalar=r01,
                            in1=xt[:, 1:W + 1], op0=Alu.mult, op1=Alu.add)
                        nc.vector.scalar_tensor_tensor(
                            out=h2[:], in0=h1[:], scalar=r12,
                            in1=xt[:, 2:W + 2], op0=Alu.mult, op1=Alu.add)
                        nc.vector.scalar_tensor_tensor(
                            out=h1[:], in0=h2[:], scalar=r21,
                            in1=xt[:, 3:W + 3], op0=Alu.mult, op1=Alu.add)
                        nc.vector.scalar_tensor_tensor(
                            out=h2[:], in0=h1[:], scalar=r10,
                            in1=xt[:, 4:W + 4], op0=Alu.mult, op1=Alu.add)
                        for lo in (0, 512):
                            nc.tensor.matmul(out=bl[:, lo:lo + 512],
                                             lhsT=bv[0:n_in, 0:n_bl],
                                             rhs=h2[:, lo:lo + 512],
                                             start=True, stop=True)
                    # copy to SBUF with 1-col zero margins
                    blt = blspool.tile([n_bl, W + 2], f32, tag="bls")
                    fence(blt)
                    nc.vector.memset(blt[:, 0:1], 0.0)
                    nc.vector.memset(blt[:, W + 1:W + 2], 0.0)
                    nc.scalar.copy(out=blt[:, 1:W + 1], in_=bl[:])
                    blm = blt[:, 0:W]
                    blc = blt[:, 1:W + 1]
                    blp = blt[:, 2:W + 2]
                    # sobel on PE: gx = B121@blm - B121@blp
                    #              gy = B10M1@(blp+blm) + 2*B10M1@blc
                    gx = psB.tile([n_gxy, W], f32, tag="gx")
                    gy = psB.tile([n_gxy, W], f32, tag="gy")
                    for lo in (0, 512):
                        nc.tensor.matmul(out=gx[:, lo:lo + 512],
                                         lhsT=b121[0:n_bl, 0:n_gxy],
                                         rhs=blm[:, lo:lo + 512],
                                         start=True, stop=False)
                        nc.tensor.matmul(out=gx[:, lo:lo + 512],
                                         lhsT=b121n[0:n_bl, 0:n_gxy],
                                         rhs=blp[:, lo:lo + 512],
                                         start=False, stop=True)
                        nc.tensor.matmul(out=gy[:, lo:lo + 512],
                                         lhsT=b10m1[0:n_bl, 0:n_gxy],
                                         rhs=blp[:, lo:lo + 512],
                                         start=True, stop=False)
                        nc.tensor.matmul(out=gy[:, lo:lo + 512],
                                         lhsT=b10m1x2[0:n_bl, 0:n_gxy],
                                         rhs=blc[:, lo:lo + 512],
                                         start=False, stop=False)
                        nc.tensor.matmul(out=gy[:, lo:lo + 512],
                                         lhsT=b10m1[0:n_bl, 0:n_gxy],
                                         rhs=blm[:, lo:lo + 512],
                                         start=False, stop=True)
                    # magnitude
                    sqx = sqpool.tile([n_gxy, W], f32, tag="sqx")
                    sqy = sqpool.tile([n_gxy, W], f32, tag="sqy")
                    nc.scalar.square(out=sqx[:], in_=gx[:])
                    nc.scalar.square(out=sqy[:], in_=gy[:])
                    m2 = sqpool.tile([n_gxy, W], f32, tag="m2")
                    nc.gpsimd.tensor_tensor(out=m2[:], in0=sqx[:], in1=sqy[:],
                                            op=Alu.add)
                    if c == 0:
                        nc.scalar.sqrt(out=gm_acc[0:n_gxy, 1:W + 1], in_=m2[:])
                    else:
                        magt = sqpool.tile([n_gxy, W], f32, tag="magt")
                        nc.scalar.sqrt(out=magt[:], in_=m2[:])
                        nc.gpsimd.tensor_tensor(
                            out=gm_acc[0:n_gxy, 1:W + 1],
                            in0=gm_acc[0:n_gxy, 1:W + 1],
                            in1=magt[:], op=Alu.add)
                    # gxs/gys accumulation (full range, base partition 0)
                    if c == 0:
                        nc.scalar.copy(out=gxs[0:n_gxy, :], in_=gx[:])
                        nc.scalar.copy(out=gys[0:n_gxy, :], in_=gy[:])
                    else:
                        nc.vector.tensor_tensor(out=gxs[0:n_gxy, :],
                                                in0=gxs[0:n_gxy, :],
                                                in1=gx[:], op=Alu.add)
                        nc.vector.tensor_tensor(out=gys[0:n_gxy, :],
                                                in0=gys[0:n_gxy, :],
                                                in1=gy[:], op=Alu.add)

                def make_masks(gxs, gys, n, shift, n_thin, j):
                    """u8 masks computed at conv frame [0:n], DMA-shifted down
                    by `shift` rows into persistent thin-frame tiles."""
                    a2 = sqpool.tile([n, W], f32, tag="sqx")
                    b2 = sqpool.tile([n, W], f32, tag="sqy")
                    nc.scalar.square(out=a2[:, :], in_=gxs[0:n, :])
                    nc.scalar.square(out=b2[:, :], in_=gys[0:n, :])
                    tmp = [msktpool.tile([n, W], u8, tag=t, name=t)
                           for t in ("tc0", "tc2", "tsm")]
                    guard(tmp[0], a2, b2)
                    guard(tmp[1], a2, b2)
                    nc.vector.scalar_tensor_tensor(
                        out=tmp[0][:], in0=a2[:], scalar=T22SQ,
                        in1=b2[:], op0=Alu.mult, op1=Alu.is_gt)
                    nc.vector.scalar_tensor_tensor(
                        out=tmp[1][:], in0=b2[:], scalar=T22SQ,
                        in1=a2[:], op0=Alu.mult, op1=Alu.is_gt)
                    ab = sqpool.tile([n, W], f32, tag="m2")
                    nc.gpsimd.tensor_tensor(out=ab[:], in0=gxs[0:n, :],
                                            in1=gys[0:n, :], op=Alu.mult)
                    guard(tmp[2], ab, ab)
                    nc.vector.tensor_scalar(out=tmp[2][:], in0=ab[:],
                                            scalar1=0.0, scalar2=None,
                                            op0=Alu.is_ge)
                    out = []
                    for t, tag in zip(tmp, ("c0", "c2", "sm")):
                        p = mskpool.tile([n_thin, W], u8, tag=f"{tag}_{j}")
                        fence(p)
                        nc.sync.dma_start(out=p[:],
                                          in_=t[shift:shift + n_thin, :])
                        out.append(p)
                    return out

                # main slab: 8 images x 3 channels
                for j in range(B):
                    gm_j = gmpool.tile([122, W + 2], f32, tag=f"gm{j}")
                    nc.vector.memset(gm_j[:, 0:1], 0.0)
                    nc.vector.memset(gm_j[:, W + 1:W + 2], 0.0)
                    gxs = gsumpool.tile([122, W], f32, tag="gxs")
                    gys = gsumpool.tile([122, W], f32, tag="gys")
                    for c in range(3):
                        xt = xpool.tile([128, W + 4], f32, tag="x")
                        fence(xt)
                        nc.vector.memset(xt[:, 0:2], 0.0)
                        nc.vector.memset(xt[:, W + 2:W + 4], 0.0)
                        nc.sync.dma_start(out=xt[:, 2:W + 2], in_=xm[3 * j + c])
                        conv_pipeline(xt, 128, mt["BV"], mt["B121"],
                                      mt["B121N"], mt["B10M1"], mt["B10M1X2"],
                                      gm_j, gxs, gys, c, 124, 122)
                    gm_tiles.append(gm_j)
                    # thin frame = conv rows 1..120 -> shift 1, 120 rows
                    masks.append(make_masks(gxs, gys, 122, 1, 120, j))

                # B8 block (own image); gm8 row p <-> img 940+p, row 84 = 0
                gm8 = gmpool.tile([85, W + 2], f32, tag="gm8self")
                nc.vector.memset(gm8[:], 0.0)
                gxs8 = gsumpool.tile([84, W], f32, tag="gxs")
                gys8 = gsumpool.tile([84, W], f32, tag="gys")
                for c in range(3):
                    xt = xpool.tile([88, W + 4], f32, tag="x")
                    fence(xt)
                    nc.vector.memset(xt[:, 0:2], 0.0)
                    nc.vector.memset(xt[:, W + 2:W + 4], 0.0)
                    nc.sync.dma_start(out=xt[:, 2:W + 2], in_=x8[c])
                    conv_pipeline(xt, 88, mt["BV8"], mt["B121_8"],
                                  mt["B121N_8"], mt["B10M1_8"],
                                  mt["B10M1X2_8"], gm8, gxs8, gys8, c, 86, 84)
                # thin8 frame = conv rows 3..83 -> shift 3, 81 rows
                m8 = make_masks(gxs8, gys8, 84, 3, 81, 8)

            # =========== B8 gm exchange ===================================
            ag_in = dpool.tile([81, W], f32, tag="ag_in")
            ag_out = dpool.tile([B * 81, W], f32, tag="ag_out")
            fence(gm8)
            nc.sync.dma_start(out=ag_in[:], in_=gm8[3:84, 1:W + 1])
            nc.gpsimd.collective_compute(
                "AllGather", Alu.bypass, replica_groups=[list(range(NC))],
                ins=[ag_in.opt()], outs=[ag_out.opt()])
            a2a_in = dpool.tile([B * 81, W], f32, tag="a2a_in")
            a2a_out = dpool.tile([B * 81, W], f32, tag="a2a_out")
            for b in range(B):
                dr, dc = DELTAS[b]
                nc.sync.dma_start(
                    out=a2a_in[81 * b:81 * (b + 1), :],
                    in_=gm8[3 + dr:84 + dr, 1 + dc:W + 1 + dc])
            nc.gpsimd.collective_compute(
                "AllToAll", Alu.bypass, replica_groups=[list(range(NC))],
                ins=[a2a_in.opt()], outs=[a2a_out.opt()])

            # =========== NMS phase (own scoped pools) ======================
            # thin frame: row p (base 0) <-> img row 118i-1+p, 120 rows.
            with (
                tc.tile_pool(name="ce", bufs=1) as cepool,
                tc.tile_pool(name="shp", bufs=1) as shpool,
                tc.tile_pool(name="cmap", bufs=2) as cpool2,
                tc.tile_pool(name="g8p", bufs=2) as g8pool,
                tc.tile_pool(name="pmap", bufs=1) as ppool,
                tc.tile_pool(name="nmst", bufs=1) as npool,
                tc.tile_pool(name="outp", bufs=2) as opool,
                tc.tile_pool(name="psC", bufs=2, space="PSUM") as psC,
            ):
                # center-aligned copies of gm (thin frame)
                ce = []
                for j in range(B):
                    fence(gm_tiles[j])
                    t = cepool.tile([120, W + 2], f32, tag=f"ce{j}")
                    nc.sync.dma_start(out=t[:], in_=gm_tiles[j][1:121, :])
                    ce.append(t)

                def build_shift(drow):
                    tiles = []
                    for j in range(B):
                        t = shpool.tile([120, W + 2], f32, tag=f"sh{j}")
                        fence(t)
                        if drow == 1:
                            nc.sync.dma_start(out=t[:],
                                              in_=gm_tiles[j][2:122, :])
                        else:
                            nc.sync.dma_start(out=t[:],
                                              in_=gm_tiles[j][0:120, :])
                        tiles.append(t)
                    return tiles

                def nms_core(b_masks, gm_b, get_in0, get_in1, n_thin,
                             bones, n_mp, out_lo, out_dram, n_out):
                    """Shared NMS tail; all tiles base partition 0."""
                    c0, c2, sm = b_masks
                    P = []
                    for k in range(4):
                        Cs = []
                        for j in (k, k + 4):
                            cj = cpool2.tile([n_thin, W], bf16, tag="c")
                            nc.vector.tensor_tensor(out=cj[:], in0=get_in0(j),
                                                    in1=get_in1(j),
                                                    op=Alu.is_gt)
                            Cs.append(cj)
                        tag = "psel" if k == 3 else f"p{k}"
                        bufs_k = 2 if k == 3 else None
                        pk = ppool.tile([n_thin, W], bf16, tag=tag,
                                        bufs=bufs_k)
                        nc.vector.tensor_tensor(out=pk[:], in0=Cs[0][:],
                                                in1=Cs[1][:],
                                                op=Alu.logical_and)
                        P.append(pk)
                    psel = P[3]
                    nc.vector.copy_predicated(out=psel[:], mask=sm[:],
                                              data=P[1][:])
                    nc.vector.copy_predicated(out=psel[:], mask=c0[:],
                                              data=P[0][:])
                    nc.vector.copy_predicated(out=psel[:], mask=c2[:],
                                              data=P[2][:])
                    strong = npool.tile([n_thin, W + 2], f32, tag="strong", bufs=2)
                    fence(strong)
                    nc.vector.memset(strong[:, 0:1], 0.0)
                    nc.vector.memset(strong[:, W + 1:W + 2], 0.0)
                    guard(strong, gm_b, psel)
                    nc.vector.scalar_tensor_tensor(
                        out=strong[:, 1:W + 1], in0=gm_b, scalar=HIGH_T,
                        in1=psel[:], op0=Alu.is_gt, op1=Alu.logical_and)
                    q = npool.tile([n_thin, W], f32, tag="q")
                    guard(q, gm_b, psel)
                    nc.vector.scalar_tensor_tensor(
                        out=q[:], in0=gm_b, scalar=LOW_T, in1=psel[:],
                        op0=Alu.is_ge, op1=Alu.logical_and)
                    mh = npool.tile([n_thin, W], f32, tag="mh")
                    nc.gpsimd.tensor_tensor(out=mh[:], in0=strong[:, 0:W],
                                            in1=strong[:, 2:W + 2],
                                            op=Alu.add)
                    nc.gpsimd.tensor_tensor(out=mh[:], in0=mh[:],
                                            in1=strong[:, 1:W + 1],
                                            op=Alu.add)
                    mp = psC.tile([n_mp, W], f32, tag="mp")
                    for lo2 in (0, 512):
                        nc.tensor.matmul(out=mp[:, lo2:lo2 + 512],
                                         lhsT=bones[0:n_thin, 0:n_mp],
                                         rhs=mh[:, lo2:lo2 + 512],
                                         start=True, stop=True)
                    ot = opool.tile([n_mp, W], f32, tag="ot")
                    guard(ot, mp, q)
                    nc.vector.scalar_tensor_tensor(
                        out=ot[:], in0=mp[:], scalar=0.5, in1=q[0:n_mp, :],
                        op0=Alu.is_ge, op1=Alu.logical_and)
                    nc.vector.memset(ot[:, 0:1], 0.0)
                    nc.vector.memset(ot[:, W - 1:W], 0.0)
                    nc.sync.dma_start(out=out_dram,
                                      in_=ot[out_lo:out_lo + n_out, :])

                def nms_b(b, shifted):
                    dr, dc = DELTAS[b]

                    def in0(j):
                        return ce[j][:, 1:W + 1]

                    def in1(j):
                        src = ce[j] if dr == 0 else shifted[j]
                        return src[:, 1 + dc:W + 1 + dc]

                    nms_core(masks[b], ce[b][:, 1:W + 1], in0, in1, 120,
                             mt["BONES"], 119, 1, outm[b], SLAB)

                for b in (0, 4):
                    nms_b(b, None)
                dn = build_shift(1)
                for b in (1, 2, 3):
                    nms_b(b, dn)
                up = build_shift(-1)
                for b in (5, 6, 7):
                    nms_b(b, up)

                # B8: own image only; operands pre-shifted via AllToAll.
                # thin8 frame: row p (base 0) <-> img 943+p, 81 rows.
                ce8 = g8pool.tile([81, W], f32, tag="ce8", bufs=1)
                nc.sync.dma_start(out=ce8[:], in_=gm8[3:84, 1:W + 1])

                def load8(dram_src, tag):
                    def get(j):
                        t = g8pool.tile([81, W], f32, tag=tag)
                        fence(t)
                        nc.sync.dma_start(
                            out=t[:], in_=dram_src[81 * j:81 * (j + 1), :])
                        return t[:]
                    return get

                nms_core(m8, ce8[:], load8(ag_out, "gp8"),
                         load8(a2a_out, "gs8"), 81,
                         mt["BONES8"], 81, 1, out8[:], B8_ROWS)

    _legalize_waits(nc)
    _CACHE["nc"] = nc
    return nc


def _legalize_waits(nc):
    """Several ISA encodings (S2S2D2_STT, HWDGE DMACopy, ...) hold only one
    embedded sync-wait, but Tile's scheduler can attach more. Hoist all
    embedded waits of multi-wait instructions into a NoOp injected just
    before them on the same engine queue (NoOps carry many waits fine)."""
    import concourse.mybir as mybir
    n = 0
    for f in nc.m.functions:
        for blk in f.blocks:
            out = []
            for ins in blk.instructions:
                si = ins.sync_info
                if (si is not None and si.on_wait is not None
                        and len(si.on_wait) > 1):
                    for w in si.on_wait:
                        nop = mybir.InstNoOp(
                            name=f"WFIX-{n}", engine=ins.engine,
                            sync_info=mybir.SyncInfo(on_wait=[w],
                                                     on_update=[]))
                        n += 1
                        out.append(nop)
                    ins.sync_info = mybir.SyncInfo(
                        on_wait=[],
                        on_update=list(si.on_update or []))
                out.append(ins)
            blk.instructions = out


def _in_maps(img):
    img = np.ascontiguousarray(img, dtype=np.float32)
    pad = np.zeros((B, 3, 5, W), np.float32)
    imgp = np.concatenate([pad, img], axis=2)  # rows shifted by +5
    maps = []
    for i in range(NC):
        r0 = SLAB * i  # padded row index of img row 118i-5
        xm_i = imgp[:, :, r0:r0 + 128, :].reshape(B * 3, 128, W)
        x8_i = img[i, :, B8_START - 8:, :]  # img rows 936..1023
        m = {"xm": np.ascontiguousarray(xm_i),
             "x8": np.ascontiguousarray(x8_i)}
        m.update(_const_mats(i))
        maps.append(m)
    return maps


def kernel(img, gauss_h=None, gauss_v=None, sobel_h=None, sobel_v=None,
           dir_f=None, connect_f=None, _want_time=False):
    from concourse.bass_utils import run_bass_kernel_spmd
    nc = _build_program()
    maps = _in_maps(np.asarray(img))
    res = run_bass_kernel_spmd(nc, maps, list(range(NC)), trace=_want_time)
    out = np.zeros((B, 1, H, W), np.float32)
    for i in range(NC):
        r = res.results[i]
        out[:, 0, SLAB * i:SLAB * (i + 1), :] = r["outm"]
        out[i, 0, B8_START:, :] = r["out8"]
    if _want_time:
        return out, res
    return out



# revision 12
# speedup vs baseline: 1.1330x; 1.1330x over previous
"""Canny edge detector on 8 Trainium2 NeuronCores (Bass/Tile).

Sharding: row slabs. Core i owns output rows [118*i, 118*(i+1)) of ALL 8
images. (The reference's flat gather at B=8 cross-wires images inside NMS:
sel_pos(b,h,w) = dirconv_b(gm_{idx(b,h,w)})(h,w), so every output pixel needs
all 8 images' gradient-magnitude maps at its rows -> shard by rows, not by
image.) The leftover band (rows 944..1023) is computed per-image on the
owning core FIRST, and the gm maps exchanged via DRAM collectives (AllGather
for plain maps, AllToAll for reader-shift-specific maps) that complete under
the main conv phase.

Precision strategy: NMS compares gm against its neighbors, and the blurred
field is smooth, so compare margins are tiny -- everything feeding gm must be
fp32-exact-ish (f32r/tf32/bf16 all flip >>840 px, the rel-err 2e-2 budget).
Matmuls instead use COMPENSATED bf16: x = hi(x) + lo(x) split (both bf16),
W = hi(W) + lo(W), and accumulate Whi@xhi + Whi@xlo + Wlo@xhi in f32 PSUM.
Max rel err ~1.5e-5 => ~100 flipped px (measured), 8x under budget, and the
PE runs at bf16 rate (4x faster than fp32 matmul). Sobel/connect band
matrices are small integers = exact in bf16 (no compensation needed).

All compute-engine APs must start at partition 0 (HW constraint, verified:
walrus birverifier rejects nonzero bases), so row re-alignment between
pipeline stages is done with SBUF->SBUF DMAs.

Engine split (cost model: DVE 1.04 ns/col, Act 0.83, Pool 0.83/eff where
eff=0.42 TT-add/mult, 0.6 stt/compare; cost ~ free-dim size only): Act gets
all 1-input ops (copies/squares/sqrt), DVE/Pool share the 2-input ops.
"""

import os

# Tile's subtile dependency tracking emits >1 embedded sync-wait on
# S2S2D2_STT instructions, which the ISA encoding cannot hold ("Too many
# sync wait commands" in codegen). With whole-tile deps the wait-absorbing
# guard ops keep every STT at <=1 embedded wait.
os.environ.setdefault("BY_DEFAULT_DISABLE_SUBTILE_DEPS", "1")

import numpy as np

H = 1024
W = 1024
B = 8
NC = 8
SLAB = 118                    # main-slab output rows per core
B8_START = SLAB * NC          # 944
B8_ROWS = H - B8_START        # 80
LOW_T, HIGH_T = 2.5, 5.0
T22SQ = float(np.float32(np.tan(np.pi / 8.0)) ** 2)

# direction index -> (dr, dc) neighbor offset of dir_f channel d
DELTAS = {0: (0, 1), 1: (1, 1), 2: (1, 0), 3: (1, -1),
          4: (0, -1), 5: (-1, -1), 6: (-1, 0), 7: (-1, 1)}


def _gauss5():
    n = np.arange(5, dtype=np.float32) - 2.0
    return np.exp(-0.5 * n * n).astype(np.float32)


def _band(n_in, n_out, offset, taps):
    """M[k, m] = taps[k - m - offset] for k-m-offset in range(len(taps))."""
    m_ = np.zeros((n_in, n_out), np.float32)
    for mm in range(n_out):
        for t, w in enumerate(taps):
            k = mm + offset + t
            if 0 <= k < n_in:
                m_[k, mm] = w
    return m_


# bf16 matrix inputs: name -> ([n_in, n_out], f32-source key, part)
# part: "hi" = bf16(M), "lo" = bf16(M - hi(M)), "x" = exact in bf16
MATS = {
    "BVH": ([128, 124], "BV", "hi"), "BVL": ([128, 124], "BV", "lo"),
    "B121B": ([124, 122], "B121", "x"), "B121NB": ([124, 122], "B121N", "x"),
    "B10M1B": ([124, 122], "B10M1", "x"),
    "B10M1X2B": ([124, 122], "B10M1X2", "x"),
    "BONESB": ([120, 119], "BONES", "x"),
    "BV8H": ([88, 86], "BV8", "hi"), "BV8L": ([88, 86], "BV8", "lo"),
    "B121B_8": ([86, 84], "B121_8", "x"),
    "B121NB_8": ([86, 84], "B121N_8", "x"),
    "B10M1B_8": ([86, 84], "B10M1_8", "x"),
    "B10M1X2B_8": ([86, 84], "B10M1X2_8", "x"),
    "BONESB_8": ([81, 81], "BONES8", "x"),
}


def _const_mats(core):
    import ml_dtypes
    bf = ml_dtypes.bfloat16
    g = _gauss5()
    g0 = float(g[0])
    f = {}
    # main slab: x/hb tile row k <-> img row a+k, a = 118i-5
    # bl row m <-> img a+2+m (124 rows); BV[k,m] = g0*g[k-m]
    f["BV"] = _band(128, 124, 0, (g0 * g).tolist())
    # gx/gy row m <-> img a+3+m = 118i-2+m (122 rows); bl k <-> a+2+k
    b121 = _band(124, 122, 0, [1.0, 2.0, 1.0])
    b10m1 = _band(124, 122, 0, [1.0, 0.0, -1.0])
    if core == 0:  # img rows -2,-1 must yield gm=0 (zero-pad semantics)
        b121[:, 0:2] = 0.0
        b10m1[:, 0:2] = 0.0
    f["B121"] = b121
    f["B121N"] = -b121
    f["B10M1"] = b10m1
    f["B10M1X2"] = 2.0 * b10m1
    # strong row k (base 0) <-> thin img row 118i-1+k
    # mp row p <-> img 118i-1+p (col 0 dummy); needs strong k = p-1,p,p+1
    bones = _band(120, 119, -1, [1.0, 1.0, 1.0])
    bones[:, 0] = 0.0
    if core == 0:
        bones[:, 1] = 0.0  # border row 0
    f["BONES"] = bones
    # B8 block: x8 row k <-> img 936+k; bl8 row m <-> img 938+m (86 rows)
    f["BV8"] = _band(88, 86, 0, (g0 * g).tolist())
    # gx8 row m <-> img 940+m (84 rows); bl8 k <-> 938+k: band k-m in {1,2,3}
    b121_8 = _band(86, 84, 1, [1.0, 2.0, 1.0])
    b10m1_8 = _band(86, 84, 1, [1.0, 0.0, -1.0])
    f["B121_8"] = b121_8
    f["B121N_8"] = -b121_8
    f["B10M1_8"] = b10m1_8
    f["B10M1X2_8"] = 2.0 * b10m1_8
    # strong8 row k (base 0) <-> img 943+k; mp8 row p <-> img 943+p
    # (col 0 dummy); needs strong8 k = p-1,p,p+1
    bones8 = _band(81, 81, -1, [1.0, 1.0, 1.0])
    bones8[:, 0] = 0.0
    bones8[:, 80] = 0.0  # border row 1023
    f["BONES8"] = bones8

    out = {}
    for name, (shp, src, part) in MATS.items():
        m = f[src]
        assert list(m.shape) == shp, (name, m.shape, shp)
        hi = m.astype(bf)
        if part == "hi":
            out[name] = np.ascontiguousarray(hi)
        elif part == "lo":
            out[name] = np.ascontiguousarray((m - hi.astype(np.float32))
                                             .astype(bf))
        else:
            assert np.array_equal(hi.astype(np.float32), m), name
            out[name] = np.ascontiguousarray(hi)
    return out


_CACHE = {}


def _build_program():
    if "nc" in _CACHE:
        return _CACHE["nc"]
    import concourse.bass as bass
    import concourse.mybir as mybir
    from concourse.tile import TileContext

    f32 = mybir.dt.float32
    bf16 = mybir.dt.bfloat16
    u8 = mybir.dt.uint8
    Alu = mybir.AluOpType

    g = _gauss5()
    r01 = float(g[0] / g[1])
    r12 = float(g[1] / g[2])
    r21 = float(g[2] / g[1])
    r10 = float(g[1] / g[0])

    nc = bass.Bass()

    xm = nc.declare_dram_parameter("xm", [B * 3, 128, W], f32, isOutput=False)
    x8 = nc.declare_dram_parameter("x8", [3, 88, W], f32, isOutput=False)
    mat_d = {k: nc.declare_dram_parameter(k, v[0], bf16, isOutput=False)
             for k, v in MATS.items()}
    outm = nc.declare_dram_parameter("outm", [B, SLAB, W], f32, isOutput=True)
    out8 = nc.declare_dram_parameter("out8", [B8_ROWS, W], f32, isOutput=True)

    with TileContext(nc) as tc:
        with (
            tc.tile_pool(name="consts", bufs=1) as cpool,
            tc.tile_pool(name="gmp", bufs=1) as gmpool,
            tc.tile_pool(name="msk", bufs=1) as mskpool,
            tc.tile_pool(name="dram", bufs=1, space="DRAM") as dpool,
        ):
            mt = {}
            for name, (shp, _, _) in MATS.items():
                t = cpool.tile(shp, bf16, tag=name)
                nc.sync.dma_start(out=t[:], in_=mat_d[name][:])
                mt[name] = t

            gm_tiles = []
            masks = []
            # =========== conv phase (own scoped pools) =====================
            with (
                tc.tile_pool(name="xin", bufs=3) as xpool,
                tc.tile_pool(name="hbt", bufs=2) as hbpool,
                tc.tile_pool(name="bls", bufs=2) as blspool,
                tc.tile_pool(name="sq", bufs=2) as sqpool,
                tc.tile_pool(name="gsum", bufs=2) as gsumpool,
                tc.tile_pool(name="mskt", bufs=2) as msktpool,
                tc.tile_pool(name="psA", bufs=2, space="PSUM") as psA,
                tc.tile_pool(name="psB", bufs=1, space="PSUM") as psB,
            ):
                def conv_pipeline(xt, n_in, sfx, gm_acc, gxs, gys, c,
                                  n_bl, n_gxy):
                    def M(base):
                        return mt[base + sfx]

                    # H-blur (Horner, 4 fused ops) -> h2 [n_in, W] f32.
                    # Last op on Pool to balance engine load.
                    h1 = hbpool.tile([n_in, W], f32, tag="h1")
                    h2 = hbpool.tile([n_in, W], f32, tag="h2")
                    nc.vector.scalar_tensor_tensor(
                        out=h1[:], in0=xt[:, 0:W], scalar=r01,
                        in1=xt[:, 1:W + 1], op0=Alu.mult, op1=Alu.add)
                    nc.vector.scalar_tensor_tensor(
                        out=h2[:], in0=h1[:], scalar=r12,
                        in1=xt[:, 2:W + 2], op0=Alu.mult, op1=Alu.add)
                    nc.vector.scalar_tensor_tensor(
                        out=h1[:], in0=h2[:], scalar=r21,
                        in1=xt[:, 3:W + 3], op0=Alu.mult, op1=Alu.add)
                    nc.gpsimd.scalar_tensor_tensor(
                        out=h2[:], in0=h1[:], scalar=r10,
                        in1=xt[:, 4:W + 4], op0=Alu.mult, op1=Alu.add)
                    # hi/lo bf16 split of h2
                    h2h = hbpool.tile([n_in, W], bf16, tag="h2h")
                    h2l = hbpool.tile([n_in, W], bf16, tag="h2l")
                    nc.scalar.copy(out=h2h[:], in_=h2[:])
                    nc.gpsimd.scalar_tensor_tensor(
                        out=h2l[:], in0=h2[:], scalar=1.0, in1=h2h[:],
                        op0=Alu.mult, op1=Alu.subtract)
                    # V-blur, compensated bf16 -> f32 PSUM
                    bl = psA.tile([n_bl, W], f32, tag="bl")
                    for lo in (0, 512):
                        nc.tensor.matmul(out=bl[:, lo:lo + 512],
                                         lhsT=M("BVH")[0:n_in, 0:n_bl],
                                         rhs=h2h[:, lo:lo + 512],
                                         start=True, stop=False)
                        nc.tensor.matmul(out=bl[:, lo:lo + 512],
                                         lhsT=M("BVH")[0:n_in, 0:n_bl],
                                         rhs=h2l[:, lo:lo + 512],
                                         start=False, stop=False)
                        nc.tensor.matmul(out=bl[:, lo:lo + 512],
                                         lhsT=M("BVL")[0:n_in, 0:n_bl],
                                         rhs=h2h[:, lo:lo + 512],
                                         start=False, stop=True)
                    # hi/lo bf16 split of bl, with 1-col zero margins
                    blh = blspool.tile([n_bl, W + 2], bf16, tag="blh")
                    bll = blspool.tile([n_bl, W + 2], bf16, tag="bll")
                    nc.vector.memset(blh[:, 0:1], 0.0)
                    nc.vector.memset(blh[:, W + 1:W + 2], 0.0)
                    nc.vector.memset(bll[:, 0:1], 0.0)
                    nc.vector.memset(bll[:, W + 1:W + 2], 0.0)
                    nc.scalar.copy(out=blh[:, 1:W + 1], in_=bl[:])
                    nc.vector.scalar_tensor_tensor(
                        out=bll[:, 1:W + 1], in0=bl[:], scalar=1.0,
                        in1=blh[:, 1:W + 1], op0=Alu.mult, op1=Alu.subtract)
                    # sobel on PE (exact bf16 weights, hi+lo operands):
                    # gx = B121@blm - B121@blp ; gy = B10M1@(blm+blp)+2@blc
                    gx = psB.tile([n_gxy, W], f32, tag="gx")
                    gy = psB.tile([n_gxy, W], f32, tag="gy")
                    for lo in (0, 512):
                        for i, (mat, off) in enumerate(
                                (("B121B", 0), ("B121NB", 2))):
                            for j, src in enumerate((blh, bll)):
                                nc.tensor.matmul(
                                    out=gx[:, lo:lo + 512],
                                    lhsT=M(mat)[0:n_bl, 0:n_gxy],
                                    rhs=src[:, lo + off:lo + off + 512],
                                    start=(i == 0 and j == 0),
                                    stop=(i == 1 and j == 1))
                        for i, (mat, off) in enumerate(
                                (("B10M1B", 2), ("B10M1X2B", 1),
                                 ("B10M1B", 0))):
                            for j, src in enumerate((blh, bll)):
                                nc.tensor.matmul(
                                    out=gy[:, lo:lo + 512],
                                    lhsT=M(mat)[0:n_bl, 0:n_gxy],
                                    rhs=src[:, lo + off:lo + off + 512],
                                    start=(i == 0 and j == 0),
                                    stop=(i == 2 and j == 1))
                    # magnitude
                    sqx = sqpool.tile([n_gxy, W], f32, tag="sqx")
                    sqy = sqpool.tile([n_gxy, W], f32, tag="sqy")
                    nc.scalar.square(out=sqx[:], in_=gx[:])
                    nc.scalar.square(out=sqy[:], in_=gy[:])
                    m2 = sqpool.tile([n_gxy, W], f32, tag="m2")
                    nc.vector.scalar_tensor_tensor(
                        out=m2[:], in0=sqx[:], scalar=1.0, in1=sqy[:],
                        op0=Alu.mult, op1=Alu.add)
                    if c == 0:
                        nc.scalar.sqrt(out=gm_acc[0:n_gxy, 1:W + 1], in_=m2[:])
                    else:
                        magt = sqpool.tile([n_gxy, W], f32, tag="magt")
                        nc.scalar.sqrt(out=magt[:], in_=m2[:])
                        nc.gpsimd.scalar_tensor_tensor(
                            out=gm_acc[0:n_gxy, 1:W + 1],
                            in0=gm_acc[0:n_gxy, 1:W + 1], scalar=1.0,
                            in1=magt[:], op0=Alu.mult, op1=Alu.add)
                    # gxs/gys accumulation (full range, base partition 0)
                    if c == 0:
                        nc.scalar.copy(out=gxs[0:n_gxy, :], in_=gx[:])
                        nc.scalar.copy(out=gys[0:n_gxy, :], in_=gy[:])
                    else:
                        nc.vector.tensor_tensor(out=gxs[0:n_gxy, :],
                                                in0=gxs[0:n_gxy, :],
                                                in1=gx[:], op=Alu.add)
                        nc.vector.tensor_tensor(out=gys[0:n_gxy, :],
                                                in0=gys[0:n_gxy, :],
                                                in1=gy[:], op=Alu.add)

                def make_masks(gxs, gys, n, shift, n_thin, j):
                    """u8 masks computed at conv frame [0:n], DMA-shifted down
                    by `shift` rows into persistent thin-frame tiles."""
                    a2 = sqpool.tile([n, W], f32, tag="sqx")
                    b2 = sqpool.tile([n, W], f32, tag="sqy")
                    nc.scalar.square(out=a2[:, :], in_=gxs[0:n, :])
                    nc.scalar.square(out=b2[:, :], in_=gys[0:n, :])
                    tmp = [msktpool.tile([n, W], u8, tag=t, name=t)
                           for t in ("tc0", "tc2", "tsm")]
                    nc.gpsimd.scalar_tensor_tensor(
                        out=tmp[0][:], in0=a2[:], scalar=T22SQ,
                        in1=b2[:], op0=Alu.mult, op1=Alu.is_gt)
                    nc.gpsimd.scalar_tensor_tensor(
                        out=tmp[1][:], in0=b2[:], scalar=T22SQ,
                        in1=a2[:], op0=Alu.mult, op1=Alu.is_gt)
                    ab = sqpool.tile([n, W], f32, tag="m2")
                    nc.vector.tensor_tensor(out=ab[:], in0=gxs[0:n, :],
                                            in1=gys[0:n, :], op=Alu.mult)
                    nc.gpsimd.tensor_scalar(out=tmp[2][:], in0=ab[:],
                                            scalar1=0.0, scalar2=None,
                                            op0=Alu.is_ge)
                    out = []
                    for t, tag in zip(tmp, ("c0", "c2", "sm")):
                        p = mskpool.tile([n_thin, W], u8, tag=f"{tag}_{j}")
                        nc.sync.dma_start(out=p[:],
                                          in_=t[shift:shift + n_thin, :])
                        out.append(p)
                    return out

                # B8 block FIRST (own image); gm8 row p <-> img 940+p,
                # row 84 = 0. Doing it first lets both collectives start
                # ~30us in and finish under the main conv phase.
                gm8 = gmpool.tile([85, W + 2], f32, tag="gm8self")
                nc.vector.memset(gm8[:], 0.0)
                gxs8 = gsumpool.tile([84, W], f32, tag="gxs")
                gys8 = gsumpool.tile([84, W], f32, tag="gys")
                for c in range(3):
                    xt = xpool.tile([88, W + 4], f32, tag="x")
                    nc.vector.memset(xt[:, 0:2], 0.0)
                    nc.vector.memset(xt[:, W + 2:W + 4], 0.0)
                    nc.sync.dma_start(out=xt[:, 2:W + 2], in_=x8[c])
                    conv_pipeline(xt, 88, "_8", gm8, gxs8, gys8, c, 86, 84)
                # thin8 frame = conv rows 3..83 -> shift 3, 81 rows
                m8 = make_masks(gxs8, gys8, 84, 3, 81, 8)

                # ======= B8 gm exchange (issued early, runs under conv) ====
                ag_in = dpool.tile([81, W], f32, tag="ag_in")
                ag_out = dpool.tile([B * 81, W], f32, tag="ag_out")
                nc.sync.dma_start(out=ag_in[:], in_=gm8[3:84, 1:W + 1])
                nc.gpsimd.collective_compute(
                    "AllGather", Alu.bypass, replica_groups=[list(range(NC))],
                    ins=[ag_in.opt()], outs=[ag_out.opt()])
                a2a_in = dpool.tile([B * 81, W], f32, tag="a2a_in")
                a2a_out = dpool.tile([B * 81, W], f32, tag="a2a_out")
                for b in range(B):
                    dr, dc = DELTAS[b]
                    nc.sync.dma_start(
                        out=a2a_in[81 * b:81 * (b + 1), :],
                        in_=gm8[3 + dr:84 + dr, 1 + dc:W + 1 + dc])
                nc.gpsimd.collective_compute(
                    "AllToAll", Alu.bypass, replica_groups=[list(range(NC))],
                    ins=[a2a_in.opt()], outs=[a2a_out.opt()])

                # main slab: 8 images x 3 channels
                for j in range(B):
                    gm_j = gmpool.tile([122, W + 2], f32, tag=f"gm{j}")
                    nc.vector.memset(gm_j[:, 0:1], 0.0)
                    nc.vector.memset(gm_j[:, W + 1:W + 2], 0.0)
                    gxs = gsumpool.tile([122, W], f32, tag="gxs")
                    gys = gsumpool.tile([122, W], f32, tag="gys")
                    for c in range(3):
                        xt = xpool.tile([128, W + 4], f32, tag="x")
                        nc.vector.memset(xt[:, 0:2], 0.0)
                        nc.vector.memset(xt[:, W + 2:W + 4], 0.0)
                        nc.sync.dma_start(out=xt[:, 2:W + 2], in_=xm[3 * j + c])
                        conv_pipeline(xt, 128, "", gm_j, gxs, gys, c, 124, 122)
                    gm_tiles.append(gm_j)
                    # thin frame = conv rows 1..120 -> shift 1, 120 rows
                    masks.append(make_masks(gxs, gys, 122, 1, 120, j))

            # =========== NMS phase (own scoped pools) ======================
            # thin frame: row p (base 0) <-> img row 118i-1+p, 120 rows.
            with (
                tc.tile_pool(name="ce", bufs=1) as cepool,
                tc.tile_pool(name="shp", bufs=1) as shpool,
                tc.tile_pool(name="cmap", bufs=2) as cpool2,
                tc.tile_pool(name="g8p", bufs=2) as g8pool,
                tc.tile_pool(name="pmap", bufs=1) as ppool,
                tc.tile_pool(name="nmst", bufs=1) as npool,
                tc.tile_pool(name="outp", bufs=2) as opool,
                tc.tile_pool(name="psC", bufs=2, space="PSUM") as psC,
            ):
                def build_shift(drow):
                    tiles = []
                    for j in range(B):
                        t = shpool.tile([120, W + 2], f32, tag=f"sh{j}")
                        if drow == 1:
                            nc.sync.dma_start(out=t[:],
                                              in_=gm_tiles[j][2:122, :])
                        else:
                            nc.sync.dma_start(out=t[:],
                                              in_=gm_tiles[j][0:120, :])
                        tiles.append(t)
                    return tiles

                def nms_core(b_masks, gm_b, get_in0, get_in1, n_thin,
                             bones, n_mp, out_lo, out_dram, n_out):
                    """Shared NMS tail; all tiles base partition 0.

                    DVE/Pool share the 12 compare/and ops; the 3x3 connect
                    conv runs on PE in exact bf16 integer arithmetic."""
                    c0, c2, sm = b_masks
                    P = []
                    for k in range(4):
                        Cs = []
                        for j in (k, k + 4):
                            cj = cpool2.tile([n_thin, W], bf16, tag="c")
                            eng = nc.gpsimd if j >= 4 else nc.vector
                            eng.tensor_tensor(out=cj[:], in0=get_in0(j),
                                              in1=get_in1(j), op=Alu.is_gt)
                            Cs.append(cj)
                        tag = "psel" if k == 3 else f"p{k}"
                        bufs_k = 2 if k == 3 else None
                        pk = ppool.tile([n_thin, W], bf16, tag=tag,
                                        bufs=bufs_k)
                        eng = nc.gpsimd if k in (0, 2) else nc.vector
                        eng.tensor_tensor(out=pk[:], in0=Cs[0][:],
                                          in1=Cs[1][:], op=Alu.logical_and)
                        P.append(pk)
                    psel = P[3]
                    nc.vector.copy_predicated(out=psel[:], mask=sm[:],
                                              data=P[1][:])
                    nc.vector.copy_predicated(out=psel[:], mask=c0[:],
                                              data=P[0][:])
                    nc.vector.copy_predicated(out=psel[:], mask=c2[:],
                                              data=P[2][:])
                    # strong in bf16 ({0,1} exact) feeds the PE connect conv
                    strong = npool.tile([n_thin, W + 2], bf16, tag="strong",
                                        bufs=2)
                    nc.vector.memset(strong[:, 0:1], 0.0)
                    nc.vector.memset(strong[:, W + 1:W + 2], 0.0)
                    nc.vector.scalar_tensor_tensor(
                        out=strong[:, 1:W + 1], in0=gm_b, scalar=HIGH_T,
                        in1=psel[:], op0=Alu.is_gt, op1=Alu.logical_and)
                    q = npool.tile([n_thin, W], f32, tag="q")
                    nc.gpsimd.scalar_tensor_tensor(
                        out=q[:], in0=gm_b, scalar=LOW_T, in1=psel[:],
                        op0=Alu.is_ge, op1=Alu.logical_and)
                    # mp = 3x3 box count of strong (incl. center; center is 0
                    # wherever `middle` holds so it cannot flip ot)
                    mp = psC.tile([n_mp, W], f32, tag="mp")
                    for lo2 in (0, 512):
                        for dc2 in (0, 1, 2):
                            nc.tensor.matmul(
                                out=mp[:, lo2:lo2 + 512],
                                lhsT=bones[0:n_thin, 0:n_mp],
                                rhs=strong[:, lo2 + dc2:lo2 + dc2 + 512],
                                start=(dc2 == 0), stop=(dc2 == 2))
                    ot = opool.tile([n_mp, W], f32, tag="ot")
                    nc.vector.scalar_tensor_tensor(
                        out=ot[:], in0=mp[:], scalar=0.5, in1=q[0:n_mp, :],
                        op0=Alu.is_ge, op1=Alu.logical_and)
                    nc.vector.memset(ot[:, 0:1], 0.0)
                    nc.vector.memset(ot[:, W - 1:W], 0.0)
                    nc.sync.dma_start(out=out_dram,
                                      in_=ot[out_lo:out_lo + n_out, :])

                def nms_b(b, shifted):
                    dr, dc = DELTAS[b]

                    def in0(j):
                        return ce[j][:, 1:W + 1]

                    def in1(j):
                        src = ce[j] if dr == 0 else shifted[j]
                        return src[:, 1 + dc:W + 1 + dc]

                    nms_core(masks[b], ce[b][:, 1:W + 1], in0, in1, 120,
                             mt["BONESB"], 119, 1, outm[b], SLAB)

                # B8 first: own image only; operands pre-shifted via
                # AllToAll; collectives completed mid-conv so these DRAM
                # loads overlap the tail of the conv phase.
                # thin8 frame: row p (base 0) <-> img 943+p, 81 rows.
                ce8 = g8pool.tile([81, W], f32, tag="ce8", bufs=1)
                nc.sync.dma_start(out=ce8[:], in_=gm8[3:84, 1:W + 1])

                def load8(dram_src, tag):
                    def get(j):
                        t = g8pool.tile([81, W], f32, tag=tag)
                        nc.sync.dma_start(
                            out=t[:], in_=dram_src[81 * j:81 * (j + 1), :])
                        return t[:]
                    return get

                nms_core(m8, ce8[:], load8(ag_out, "gp8"),
                         load8(a2a_out, "gs8"), 81,
                         mt["BONESB_8"], 81, 1, out8[:], B8_ROWS)

                # center-aligned copies of gm (thin frame)
                ce = []
                for j in range(B):
                    t = cepool.tile([120, W + 2], f32, tag=f"ce{j}")
                    nc.sync.dma_start(out=t[:], in_=gm_tiles[j][1:121, :])
                    ce.append(t)

                for b in (0, 4):
                    nms_b(b, None)
                dn = build_shift(1)
                for b in (1, 2, 3):
                    nms_b(b, dn)
                up = build_shift(-1)
                for b in (5, 6, 7):
                    nms_b(b, up)

    _legalize_waits(nc)
    _CACHE["nc"] = nc
    return nc


def _legalize_waits(nc):
    """Several ISA encodings (S2S2D2_STT, HWDGE DMACopy, ...) hold only one
    embedded sync-wait, but Tile's scheduler can attach more. Hoist all
    embedded waits of multi-wait instructions into a NoOp injected just
    before them on the same engine queue (NoOps carry many waits fine)."""
    import concourse.mybir as mybir
    n = 0
    for f in nc.m.functions:
        for blk in f.blocks:
            out = []
            for ins in blk.instructions:
                si = ins.sync_info
                if (si is not None and si.on_wait is not None
                        and len(si.on_wait) > 1):
                    for w in si.on_wait:
                        nop = mybir.InstNoOp(
                            name=f"WFIX-{n}", engine=ins.engine,
                            sync_info=mybir.SyncInfo(on_wait=[w],
                                                     on_update=[]))
                        n += 1
                        out.append(nop)
                    ins.sync_info = mybir.SyncInfo(
                        on_wait=[],
                        on_update=list(si.on_update or []))
                out.append(ins)
            blk.instructions = out


def _in_maps(img):
    img = np.ascontiguousarray(img, dtype=np.float32)
    pad = np.zeros((B, 3, 5, W), np.float32)
    imgp = np.concatenate([pad, img], axis=2)  # rows shifted by +5
    maps = []
    for i in range(NC):
        r0 = SLAB * i  # padded row index of img row 118i-5
        xm_i = imgp[:, :, r0:r0 + 128, :].reshape(B * 3, 128, W)
        x8_i = img[i, :, B8_START - 8:, :]  # img rows 936..1023
        m = {"xm": np.ascontiguousarray(xm_i),
             "x8": np.ascontiguousarray(x8_i)}
        m.update(_const_mats(i))
        maps.append(m)
    return maps


def kernel(img, gauss_h=None, gauss_v=None, sobel_h=None, sobel_v=None,
           dir_f=None, connect_f=None, _want_time=False):
    from concourse.bass_utils import run_bass_kernel_spmd
    nc = _build_program()
    maps = _in_maps(np.asarray(img))
    res = run_bass_kernel_spmd(nc, maps, list(range(NC)), trace=_want_time)
    out = np.zeros((B, 1, H, W), np.float32)
    for i in range(NC):
        r = res.results[i]
        out[:, 0, SLAB * i:SLAB * (i + 1), :] = r["outm"]
        out[i, 0, B8_START:, :] = r["out8"]
    if _want_time:
        return out, res
    return out
